# revision 37
# baseline (speedup 1.0000x reference)
"""AlternatingDiffHead Trainium2 kernel.

Data-parallel over batch: B=8 batch elements -> 8 NeuronCores, one batch
element per core, no collectives.

Per-core math (T=2048, C=1024, HS=128, 2 terms):
  v  = x @ Wv                                  [T, 256]
  qn = rope(x @ Wqn * 1/sqrt(HS)),  kn = rope(x @ Wkn)     [T, 128]
  Sn = qn @ kn^T  (causal)                      [T, T]
  En = exp(Sn)    (no max-sub; S is O(1))       rowsum -> ln
  D  = E0 + beta E1,  beta[t] = (c1 l0[t]) / (c0 l1[t])
  out[t] = (c0 / l0[t]) * (D @ v)[t]
where c0 = lam0, c1 = -lam1 (host-computed scalars).

Layout / engine choices:
 - x passed transposed [C, T] (c on partitions); all inputs land via a
   handful of consolidated multi-dim-AP DMAs ordered by compute need
   (wqk + x t-block 0 first) so the PE starts ~13us in instead of ~36us.
 - q/k head dim permuted to rotate-half order on host; RoPE pair swap is
   two SBUF-SBUF DMAs (64-partition rotation), issued on the sync DGE.
 - causal mask applied inside PSUM by one extra bf16 matmul per
   diagonal block: S += I^T @ (-30 * triu) => exp gives ~1e-13.
 - D-combine fused into one DVE pass via scalar_tensor_tensor.
 - D^T for the AV matmul via PE transpose-mode, group-pipelined:
   transpose groups g and g+1 are issued back-to-back so the PSUM->SBUF
   copy of group g (alternating scalar/vector engines) hides behind the
   transposes of g+1 before the AV matmuls consume them.
 - all matmuls bf16 (fp32 PSUM accumulation).
 - emission software-pipelined: attention rows of block tt interleave
   with projections of block tt+1; within a block rows go largest-first
   to shorten the serial tail.
"""

import numpy as np
import ml_dtypes
from contextlib import ExitStack

import concourse.bass as bass
import concourse.tile as tile
from concourse import bacc, mybir

B, T, C, HS, NT = 8, 2048, 1024, 128, 2
E2 = 2 * HS  # v/out feature dim (256)
THETA = 10000.0
NEG = -30.0
BF16, F32 = mybir.dt.bfloat16, mybir.dt.float32
AF = mybir.ActivationFunctionType
ALU = mybir.AluOpType
NCC = C // 128         # 8 contraction chunks
NTILE = T // 128       # 16 row tiles
NTT = 4                # 512-wide t-blocks
SCHUNK = 1024          # attention-score chunk width (2 PSUM banks)


def build_nc():
    nc = bacc.Bacc("TRN2", target_bir_lowering=False, debug=False, num_devices=8)

    xT = nc.declare_dram_parameter("xT", [C, T], BF16, isOutput=False)
    wqk = nc.declare_dram_parameter("wqk", [C, 4 * HS], BF16, isOutput=False)
    wv = nc.declare_dram_parameter("wv", [C, E2], BF16, isOutput=False)
    cosb = nc.declare_dram_parameter("cosb", [HS, T], BF16, isOutput=False)
    sinb = nc.declare_dram_parameter("sinb", [HS, T], BF16, isOutput=False)
    cmask = nc.declare_dram_parameter("cmask", [HS, 2 * HS], BF16, isOutput=False)
    lamc = nc.declare_dram_parameter("lamc", [HS, 2], F32, isOutput=False)
    outp = nc.declare_dram_parameter("out", [T, E2], F32, isOutput=True)

    with tile.TileContext(nc) as tc:
        with ExitStack() as ctx:
            pers = ctx.enter_context(tc.tile_pool(name="pers", bufs=1))
            pproj = ctx.enter_context(
                tc.tile_pool(name="pproj", bufs=2, space="PSUM")
            )
            ps = ctx.enter_context(tc.tile_pool(name="ps", bufs=2, space="PSUM"))
            pdt = ctx.enter_context(tc.tile_pool(name="pdt", bufs=1, space="PSUM"))
            pav = ctx.enter_context(tc.tile_pool(name="pav", bufs=1, space="PSUM"))
            rp = ctx.enter_context(tc.tile_pool(name="rope", bufs=3))
            ep = ctx.enter_context(tc.tile_pool(name="ep", bufs=3))
            dp = ctx.enter_context(tc.tile_pool(name="dp", bufs=2))
            dtp = ctx.enter_context(tc.tile_pool(name="dts", bufs=3))
            st = ctx.enter_context(tc.tile_pool(name="st", bufs=4))
            op = ctx.enter_context(tc.tile_pool(name="op", bufs=3))

            xt_s = pers.tile([128, NCC, T], BF16)          # [p, c, t]
            wqk_s = pers.tile([128, NCC, 4 * HS], BF16)    # [p, c, 4hs]
            wv_s = pers.tile([128, NCC, E2], BF16)         # [p, c, e]
            cos_s = pers.tile([128, T], BF16)
            sin_s = pers.tile([128, T], BF16)
            msk_s = pers.tile([128, 2 * HS], BF16)         # [I | -30*triu]
            lam_s = pers.tile([128, 2], F32)               # [c0, c1/c0]
            v_t = [
                pers.tile([128, E2], BF16, name=f"v{j}", tag=f"v{j}")
                for j in range(NTILE)
            ]
            # qr tiles: tensor tau (0=q0 1=q1 2=k0 3=k1) x 512-col block
            qr_t = [
                [
                    pers.tile([128, 512], BF16, name=f"qr{t}_{s}", tag=f"qr{t}_{s}")
                    for s in range(NTT)
                ]
                for t in range(4)
            ]

            # consolidated input DMAs, ordered by when compute needs them
            xT3 = xT[:].rearrange("(c p) t -> p c t", p=128)
            wqk3 = wqk[:].rearrange("(c p) d -> p c d", p=128)
            # wqk on the sync DGE, x on the scalar DGE: the two first
            # transfers stream in parallel so the first proj matmul can
            # start ~1.5us earlier
            nc.sync.dma_start(wqk_s[:, 0:4, :], wqk3[:, 0:4, :])
            nc.scalar.dma_start(xt_s[:, 0:4, 0:512], xT3[:, 0:4, 0:512])
            nc.sync.dma_start(wqk_s[:, 4:8, :], wqk3[:, 4:8, :])
            nc.scalar.dma_start(xt_s[:, 4:8, 0:512], xT3[:, 4:8, 0:512])
            nc.sync.dma_start(cos_s[:, 0:1024], cosb[:, 0:1024])
            nc.sync.dma_start(sin_s[:, 0:1024], sinb[:, 0:1024])
            nc.sync.dma_start(msk_s[:], cmask[:])
            nc.sync.dma_start(lam_s[:], lamc[:])
            nc.sync.dma_start(
                wv_s[:], wv[:].rearrange("(c p) d -> p c d", p=128)
            )
            nc.sync.dma_start(xt_s[:, :, 512:1024], xT3[:, :, 512:1024])
            nc.sync.dma_start(xt_s[:, :, 1024:1536], xT3[:, :, 1024:1536])
            nc.sync.dma_start(xt_s[:, :, 1536:2048], xT3[:, :, 1536:2048])
            nc.sync.dma_start(cos_s[:, 1024:2048], cosb[:, 1024:2048])
            nc.sync.dma_start(sin_s[:, 1024:2048], sinb[:, 1024:2048])
            i_ap = msk_s[:, 0:128]
            u_ap = msk_s[:, 128:256]

            def proj_qk(tau, tt):
                sl = slice(512 * tt, 512 * (tt + 1))
                qp = pproj.tile([128, 512], F32, tag="pj")
                for c in range(NCC):
                    nc.tensor.matmul(
                        qp[:],
                        wqk_s[:, c, 128 * tau : 128 * (tau + 1)],
                        xt_s[:, c, sl],
                        start=(c == 0),
                        stop=(c == NCC - 1),
                    )
                qb = rp.tile([128, 512], BF16, tag="qb")
                if tt == 0:
                    # vector is idle before the first attention rows; the
                    # scalar queue would delay these casts behind nothing
                    # useful and stall the pproj buffer rotation
                    nc.vector.tensor_copy(qb[:], qp[:])
                else:
                    nc.scalar.activation(qb[:], qp[:], AF.Copy)
                qsw = rp.tile([128, 512], BF16, tag="qsw")
                nc.sync.dma_start(qsw[0:64, :], qb[64:128, :])
                nc.sync.dma_start(qsw[64:128, :], qb[0:64, :])
                t1 = rp.tile([128, 512], BF16, tag="t1")
                nc.vector.tensor_mul(t1[:], qb[:], cos_s[:, sl])
                t2 = rp.tile([128, 512], BF16, tag="t2")
                nc.vector.tensor_mul(t2[:], qsw[:], sin_s[:, sl])
                nc.gpsimd.tensor_add(qr_t[tau][tt][:], t1[:], t2[:])

            def proj_v(j):
                vp = pproj.tile([128, 512], F32, tag="pj")
                for c in range(NCC):
                    nc.tensor.matmul(
                        vp[:, :E2],
                        xt_s[:, c, 128 * j : 128 * (j + 1)],
                        wv_s[:, c, :],
                        start=(c == 0),
                        stop=(c == NCC - 1),
                    )
                nc.vector.tensor_copy(v_t[j][:], vp[:, :E2])

            def attention(i):
                W = 128 * (i + 1)
                nch = (W + SCHUNK - 1) // SCHUNK
                es, ls = [], []
                for n in range(2):
                    en = ep.tile([128, T], BF16, tag=f"E{n}")
                    lp = st.tile([128, 4], F32, tag=f"lp{n}")
                    for ch in range(nch):
                        off = SCHUNK * ch
                        wch = min(SCHUNK, W - off)
                        sp = ps.tile([128, SCHUNK], F32, tag="sp")
                        diag = off + wch == W
                        for sub in range(0, wch, 512):
                            wsub = min(512, wch - sub)
                            so = off + sub
                            nc.tensor.matmul(
                                sp[:, sub : sub + wsub],
                                qr_t[n][i // 4][
                                    :, 128 * (i % 4) : 128 * (i % 4 + 1)
                                ],
                                qr_t[2 + n][so // 512][:, :wsub],
                                start=True,
                                stop=not (diag and sub + wsub == wch),
                                skip_group_check=True,
                            )
                        if diag:
                            nc.tensor.matmul(
                                sp[:, wch - 128 : wch],
                                i_ap,
                                u_ap,
                                start=False,
                                stop=True,
                                skip_group_check=True,
                            )
                        nc.scalar.activation(
                            en[:, off : off + wch],
                            sp[:, :wch],
                            AF.Exp,
                            accum_out=lp[:, ch : ch + 1],
                        )
                    if nch == 1:
                        ln_ap = lp[:, 0:1]
                    else:
                        ln = st.tile([128, 1], F32, tag=f"l{n}")
                        nc.vector.tensor_reduce(
                            ln[:], lp[:, :nch], mybir.AxisListType.X, ALU.add
                        )
                        ln_ap = ln[:]
                    es.append(en)
                    ls.append(ln_ap)

                r1 = st.tile([128, 1], F32, tag="r1")
                nc.vector.reciprocal(r1[:], ls[1])
                beta = st.tile([128, 1], F32, tag="beta")
                nc.vector.tensor_scalar(
                    beta[:], ls[0], r1[:], lam_s[:, 1:2], ALU.mult, ALU.mult
                )

                # D = E1 * beta + E0, fused DVE pass per 1024-chunk so the
                # first transpose group starts before the full row combines
                d = dp.tile([128, T], BF16, tag="d")
                for off in range(0, W, SCHUNK):
                    wch = min(SCHUNK, W - off)
                    nc.vector.scalar_tensor_tensor(
                        d[:, off : off + wch],
                        es[1][:, off : off + wch],
                        beta[:],
                        es[0][:, off : off + wch],
                        ALU.mult, ALU.add,
                    )

                # alpha only gates the final ot scale; emit it after the
                # combine so it doesn't delay the STT in the vector FIFO
                r0 = st.tile([128, 1], F32, tag="r0")
                nc.vector.reciprocal(r0[:], ls[0])
                alpha = st.tile([128, 1], F32, tag="alpha")
                nc.vector.tensor_mul(alpha[:], r0[:], lam_s[:, 0:1])

                # group-pipelined PE transpose: emit T(g) for all groups
                # first (pdt bufs=2), copies alternate scalar/vector so
                # they hide behind the next group's transposes.
                ngrp = (i + 1 + 7) // 8
                dt_sbs = []
                for g in range(ngrp):
                    jb = 8 * g
                    nb = min(8, i + 1 - jb)
                    dt_ps = pdt.tile([128, 1024], BF16, tag="dtps")
                    for m in range(nb):
                        nc.tensor.transpose(
                            dt_ps[:, 128 * m : 128 * (m + 1)],
                            d[:, 128 * (jb + m) : 128 * (jb + m + 1)],
                            i_ap,
                        )
                    dt_sb = dtp.tile([128, 1024], BF16, name=f"dtsb{g}", tag=f"dtsb{g % 3}")
                    if g % 2 == 0:
                        nc.vector.tensor_copy(dt_sb[:, : 128 * nb], dt_ps[:, : 128 * nb])
                    else:
                        nc.scalar.activation(
                            dt_sb[:, : 128 * nb], dt_ps[:, : 128 * nb], AF.Copy
                        )
                    dt_sbs.append(dt_sb)

                av = pav.tile([128, E2], F32, tag="av")
                for g in range(ngrp):
                    jb = 8 * g
                    nb = min(8, i + 1 - jb)
                    for m in range(nb):
                        j = jb + m
                        nc.tensor.matmul(
                            av[:],
                            dt_sbs[g][:, 128 * m : 128 * (m + 1)],
                            v_t[j][:],
                            start=(j == 0),
                            stop=(j == i),
                            skip_group_check=True,
                        )

                ot = op.tile([128, E2], F32, tag="ot")
                nc.vector.tensor_scalar(ot[:], av[:], alpha[:], None, ALU.mult)
                nc.sync.dma_start(outp[128 * i : 128 * (i + 1), :], ot[:])

            # software-pipelined emission: attention rows of block tt are
            # interleaved with the projections of block tt+1 so the PE
            # stays dense and the engine queues mix short/long ops.
            # Within a block rows go largest-first to shorten the tail.
            for tau in range(4):
                proj_qk(tau, 0)
            for j in range(4):
                proj_v(j)
            for tt in range(NTT):
                attention(4 * tt + 3)
                if tt < NTT - 1:
                    proj_qk(0, tt + 1)
                    proj_qk(1, tt + 1)
                attention(4 * tt + 2)
                if tt < NTT - 1:
                    proj_qk(2, tt + 1)
                    proj_qk(3, tt + 1)
                attention(4 * tt + 1)
                if tt < NTT - 1:
                    for j in range(4 * tt + 4, 4 * tt + 8):
                        proj_v(j)
                attention(4 * tt)

    nc.compile()
    return nc


_CACHE = {}


def _get_nc():
    if "nc" not in _CACHE:
        _CACHE["nc"] = build_nc()
    return _CACHE["nc"]


def _prep_host(x, Wq, Wk, Wv, lambda_q, lambda_k, layer_idx):
    bf = ml_dtypes.bfloat16
    perm = np.concatenate([np.arange(0, HS, 2), np.arange(1, HS, 2)])
    scale = 1.0 / np.sqrt(HS)
    Wqp = np.asarray(Wq, np.float32)[:, :, perm] * scale
    Wkp = np.asarray(Wk, np.float32)[:, :, perm]
    wqk = np.concatenate([Wqp[0], Wqp[1], Wkp[0], Wkp[1]], axis=1).astype(bf)
    wv = np.asarray(Wv, np.float32).astype(bf)

    f = 1.0 / THETA ** (np.arange(0, HS, 2, dtype=np.float64) / HS)
    ang = np.outer(f, np.arange(T, dtype=np.float64))  # [64, T]
    cosb = np.concatenate([np.cos(ang), np.cos(ang)], 0).astype(bf)
    sinb = np.concatenate([-np.sin(ang), np.sin(ang)], 0).astype(bf)

    eye = np.eye(HS, dtype=np.float32)
    u30 = NEG * np.triu(np.ones((HS, HS), np.float32), k=1)
    cmask = np.concatenate([eye, u30], axis=1).astype(bf)

    li = float(np.asarray(layer_idx))
    lam_init = 0.8 - 0.6 * np.exp(-0.3 * (li - 1.0))
    e = np.mean(
        np.exp(np.asarray(lambda_q, np.float32) * np.asarray(lambda_k, np.float32)),
        axis=-1,
    )
    lam = e - np.concatenate([[0.0], e[:-1]]) + lam_init
    c0, c1 = float(lam[0]), float(-lam[1])
    lamc = np.tile(np.array([[c0, c1 / c0]], np.float32), (HS, 1))

    xT = np.ascontiguousarray(np.asarray(x, np.float32).transpose(0, 2, 1)).astype(bf)
    return xT, wqk, wv, cosb, sinb, cmask, lamc


def kernel(x, Wq, Wk, Wv, lambda_q, lambda_k, layer_idx):
    from concourse.bass_utils import run_bass_kernel_spmd

    xT, wqk, wv, cosb, sinb, cmask, lamc = _prep_host(
        x, Wq, Wk, Wv, lambda_q, lambda_k, layer_idx
    )
    in_maps = [
        {
            "xT": xT[b],
            "wqk": wqk,
            "wv": wv,
            "cosb": cosb,
            "sinb": sinb,
            "cmask": cmask,
            "lamc": lamc,
        }
        for b in range(B)
    ]
    res = run_bass_kernel_spmd(_get_nc(), in_maps, core_ids=list(range(B)))
    return np.stack([res.results[b]["out"] for b in range(B)]).astype(np.float32)
